# revision 11
# baseline (speedup 1.0000x reference)
"""ConvBERT attention block (SeparableConv1D key + dynamic conv) on 8 TRN2 NeuronCores.

Sharding: data-parallel over batch (B=8 -> 1 sample per core), weights replicated.

Per-core dataflow (all activations in [C, L] layout, channels on partitions):
  xT [768, 2048] bf16 (host-pretransposed)
  conv: dw_out[c, l] = sum_k xT[c, l+k-4] * dw[c, k]   (PE diag-matmuls + DVE MACs)
  q|co = W_qco^T @ xT        (TensorE, K=C contraction, out [o, l])
  key  = pw^T @ dw_out       (TensorE)
  attn = key * q             (DVE)
  kernT_pre = W_ck^T @ attn  -> [108, L] logits, exp on ACT (softmax w/o max: logits are tiny)
  sums = ones_block @ expT   -> [12, L]; recip on DVE
  einsum: out[c, l] = (sum_k co[c, l+k-4] * expT[h(c)*9+k, l]) * recip[h(c), l]
          (DVE: 1 windowed 9-tap mult + in-place tree adds + norm mult)
  out [768, 2048] bf16 -> host transposes back to [L, C] f32.
"""

import os
import sys

for _p in ("/opt/trn_rl_repo", "/root/.axon_site/_ro/trn_rl_repo"):
    if os.path.isdir(_p) and _p not in sys.path:
        sys.path.append(_p)

import ml_dtypes
import numpy as np

import concourse.bass as bass
import concourse.mybir as mybir
import concourse.tile as tile
from concourse import bacc
from concourse.bass_utils import run_bass_kernel_spmd
from concourse.masks import make_identity

BF16 = mybir.dt.bfloat16
F32 = mybir.dt.float32

H, D, K = 12, 64, 9
C = H * D  # 768
L = 2048
B = 8
PAD = (K - 1) // 2  # 4
P = 128
NCT = C // P  # 6 channel tiles
LC = 512  # l-chunk (one PSUM bank of f32)
NLC = L // LC  # 4
HK = H * K  # 108
PE_CONV_CT = 3  # ctiles [0, PE_CONV_CT) do depthwise conv on TensorE, rest on DVE

AF = mybir.ActivationFunctionType
OP = mybir.AluOpType


def _emit(nc, tc):
    from contextlib import ExitStack

    with ExitStack() as ctx:
        big = ctx.enter_context(tc.tile_pool(name="big", bufs=26))
        wqp = ctx.enter_context(tc.tile_pool(name="wqp", bufs=NCT))
        pwp = ctx.enter_context(tc.tile_pool(name="pwp", bufs=NCT))
        ckp = ctx.enter_context(tc.tile_pool(name="ckp", bufs=NCT))
        dwp = ctx.enter_context(tc.tile_pool(name="dwp", bufs=NCT))
        dgp = ctx.enter_context(tc.tile_pool(name="dgp", bufs=PE_CONV_CT * K + 1))
        kxp = ctx.enter_context(tc.tile_pool(name="kxp", bufs=3))
        prp = ctx.enter_context(tc.tile_pool(name="prp", bufs=3))
        onp = ctx.enter_context(tc.tile_pool(name="onp", bufs=1))
        on2 = ctx.enter_context(tc.tile_pool(name="on2", bufs=1))
        psp = ctx.enter_context(tc.tile_pool(name="psp", bufs=6, space="PSUM"))

        xT_d = nc.dram_tensor("xT", [C, L], BF16, kind="ExternalInput")
        wqco_d = nc.dram_tensor("wqco", [C, 2 * C], BF16, kind="ExternalInput")
        pwT_d = nc.dram_tensor("pwT", [C, C], BF16, kind="ExternalInput")
        wck_d = nc.dram_tensor("wck", [C, HK], BF16, kind="ExternalInput")
        dws_d = nc.dram_tensor("dws", [C, K], F32, kind="ExternalInput")
        bqco_d = nc.dram_tensor("bqco", [P, 2 * NCT], F32, kind="ExternalInput")
        bsep_d = nc.dram_tensor("bsep", [P, NCT], F32, kind="ExternalInput")
        bck_d = nc.dram_tensor("bck", [HK, 1], F32, kind="ExternalInput")
        out_d = nc.dram_tensor("out", [C, L], BF16, kind="ExternalOutput")
        expT_dram = nc.dram_tensor("expTd", [HK, L], BF16)
        recipT_dram = nc.dram_tensor("recipTd", [H, L], BF16)

        # ---- weights / constants ----
        wqco = [wqp.tile([P, 2 * C], BF16, tag="wq", name=f"wqco{i}") for i in range(NCT)]
        pwT = [pwp.tile([P, C], BF16, tag="pw", name=f"pwT{i}") for i in range(NCT)]
        wck = [ckp.tile([P, HK], BF16, tag="ck", name=f"wck{i}") for i in range(NCT)]
        dws = [dwp.tile([P, K], F32, tag="dw", name=f"dws{i}") for i in range(NCT)]
        for g in range(NCT):
            sl = slice(g * P, (g + 1) * P)
            nc.sync.dma_start(wqco[g][:], wqco_d[sl, :])
            nc.sync.dma_start(pwT[g][:], pwT_d[sl, :])
            nc.sync.dma_start(wck[g][:], wck_d[sl, :])
            nc.sync.dma_start(dws[g][:], dws_d[sl, :])
        bqco = onp.tile([P, 2 * NCT], F32, tag="bq")
        bsep = onp.tile([P, NCT], F32, tag="bs")
        bck = onp.tile([HK, 1], F32, tag="bk")
        nc.sync.dma_start(bqco[:], bqco_d[:])
        nc.sync.dma_start(bsep[:], bsep_d[:])
        nc.sync.dma_start(bck[:], bck_d[:])

        ident = dgp.tile([P, P], BF16, tag="dg")
        make_identity(nc, ident[:])
        # ones_block[p, h] = 1 iff p // 9 == h  (for summing exp over k)
        ones = on2.tile([HK, H], BF16, tag="on")
        nc.gpsimd.memset(ones[:], 1.0)
        nc.gpsimd.affine_select(
            out=ones[:], in_=ones[:], compare_op=OP.is_ge, fill=0.0,
            base=0, pattern=[[-K, H]], channel_multiplier=1)
        nc.gpsimd.affine_select(
            out=ones[:], in_=ones[:], compare_op=OP.is_ge, fill=0.0,
            base=K - 1, pattern=[[K, H]], channel_multiplier=-1)

        diag = {}
        for g in range(PE_CONV_CT):
            for k in range(K):
                d = dgp.tile([P, P], BF16, tag="dg", name=f"diag{g}_{k}")
                nc.vector.tensor_scalar_mul(d[:], ident[:], dws[g][:, k : k + 1])
                diag[(g, k)] = d

        # ---- x load (padded for conv halo) ----
        xT = []
        for g in range(NCT):
            t = big.tile([P, L + 2 * PAD], BF16, tag="big", name=f"xT{g}")
            nc.gpsimd.memset(t[:, 0:PAD], 0.0)
            nc.gpsimd.memset(t[:, L + PAD : L + 2 * PAD], 0.0)
            nc.sync.dma_start(t[:, PAD : L + PAD], xT_d[g * P : (g + 1) * P, :])
            xT.append(t)

        # ---- depthwise conv -> dwout [C, L] ----
        dwout = []
        for g in range(NCT):
            t = big.tile([P, L], BF16, tag="big", name=f"dwout{g}")
            dwout.append(t)
        for g in range(PE_CONV_CT):  # TensorE: 9 accumulating diag matmuls
            for oc in range(NLC):
                ps = psp.tile([P, LC], F32, tag="ps", name="ps")
                for k in range(K):
                    nc.tensor.matmul(
                        ps[:], diag[(g, k)][:],
                        xT[g][:, oc * LC + k : oc * LC + k + LC],
                        start=(k == 0), stop=(k == K - 1))
                nc.scalar.copy(dwout[g][:, oc * LC : (oc + 1) * LC], ps[:])
        for g in range(PE_CONV_CT, NCT):  # DVE: per-partition-scalar MAC chain
            nc.vector.tensor_scalar_mul(dwout[g][:], xT[g][:, 0:L], dws[g][:, 0:1])
            for k in range(1, K):
                nc.vector.scalar_tensor_tensor(
                    out=dwout[g][:], in0=xT[g][:, k : k + L],
                    scalar=dws[g][:, k : k + 1], in1=dwout[g][:],
                    op0=OP.mult, op1=OP.add)

        # ---- q | co projections (fused): out[o, l] = sum_c W[c, o] * xT[c, l] ----
        q = [big.tile([P, L], BF16, tag="big", name=f"q{i}") for i in range(NCT)]
        co = []
        for g in range(NCT):
            t = big.tile([P, L + 2 * PAD], BF16, tag="big", name=f"co{g}")
            nc.gpsimd.memset(t[:, 0:PAD], 0.0)
            nc.gpsimd.memset(t[:, L + PAD : L + 2 * PAD], 0.0)
            co.append(t)
        for ot in range(2 * NCT):
            for oc in range(NLC):
                ps = psp.tile([P, LC], F32, tag="ps", name="ps")
                for g in range(NCT):
                    nc.tensor.matmul(
                        ps[:], wqco[g][:, ot * P : (ot + 1) * P],
                        xT[g][:, PAD + oc * LC : PAD + (oc + 1) * LC],
                        start=(g == 0), stop=(g == NCT - 1))
                if ot < NCT:
                    dst = q[ot][:, oc * LC : (oc + 1) * LC]
                else:
                    dst = co[ot - NCT][:, PAD + oc * LC : PAD + (oc + 1) * LC]
                nc.scalar.activation(dst, ps[:], AF.Identity,
                                     bias=bqco[:, ot : ot + 1])

        # ---- key = pw^T @ dwout ----
        key = [big.tile([P, L], BF16, tag="big", name=f"key{i}") for i in range(NCT)]
        for ot in range(NCT):
            for oc in range(NLC):
                ps = psp.tile([P, LC], F32, tag="ps", name="ps")
                for g in range(NCT):
                    nc.tensor.matmul(
                        ps[:], pwT[g][:, ot * P : (ot + 1) * P],
                        dwout[g][:, oc * LC : (oc + 1) * LC],
                        start=(g == 0), stop=(g == NCT - 1))
                nc.scalar.activation(key[ot][:, oc * LC : (oc + 1) * LC], ps[:],
                                     AF.Identity, bias=bsep[:, ot : ot + 1])

        # ---- attn = key * q (in-place into key) ----
        for g in range(NCT):
            nc.vector.tensor_mul(key[g][:], key[g][:], q[g][:])

        # ---- kern logits -> expT [108, L] (unnormalized softmax numerator) ----
        expT = onp.tile([HK, L], BF16, tag="ex")
        for oc in range(NLC):
            ps = psp.tile([HK, LC], F32, tag="ps", name="psk")
            for g in range(NCT):
                nc.tensor.matmul(
                    ps[:], wck[g][:], key[g][:, oc * LC : (oc + 1) * LC],
                    start=(g == 0), stop=(g == NCT - 1))
            nc.scalar.activation(expT[:, oc * LC : (oc + 1) * LC], ps[:],
                                 AF.Exp, bias=bck[:, 0:1])
            nc.sync.dma_start(expT_dram[:, oc * LC : (oc + 1) * LC],
                              expT[:, oc * LC : (oc + 1) * LC])

        # ---- per-(h, l) softmax denominators -> recipT [12, L] ----
        recipT = onp.tile([H, L], BF16, tag="rc")
        for oc in range(NLC):
            ps = psp.tile([H, LC], F32, tag="ps", name="pss")
            nc.tensor.matmul(ps[:], ones[:], expT[:, oc * LC : (oc + 1) * LC],
                             start=True, stop=True)
            with nc.allow_low_precision(reason="bf16 softmax denominators"):
                nc.vector.reciprocal(recipT[:, oc * LC : (oc + 1) * LC], ps[:])
            nc.sync.dma_start(recipT_dram[:, oc * LC : (oc + 1) * LC],
                              recipT[:, oc * LC : (oc + 1) * LC])

        # ---- broadcast exp/recip across the 64 d-partitions of each head ----
        recip_b = []
        for g in range(NCT):
            t = big.tile([P, L], BF16, tag="big", name=f"recipb{g}")
            rb = recipT_dram[:]
            for hh in range(2):
                sap = bass.AP(rb.tensor, (2 * g + hh) * L, [[0, 64], [1, L]])
                nc.sync.dma_start(t[hh * 64 : (hh + 1) * 64, :], sap)
            recip_b.append(t)

        # ---- dynamic conv einsum + normalization ----
        out_cl = [big.tile([P, L], BF16, tag="big", name=f"outcl{i}") for i in range(NCT)]
        for g in range(NCT):
            for oc in range(NLC):
                kx = kxp.tile([P, K, LC], BF16, tag="kx", name=f"kx{g}_{oc}")
                eb = expT_dram[:]
                for hh in range(2):
                    sap = bass.AP(eb.tensor,
                                  K * (2 * g + hh) * L + oc * LC,
                                  [[0, 64], [L, K], [1, LC]])
                    nc.sync.dma_start(kx[hh * 64 : (hh + 1) * 64, :, :], sap)

                pr = prp.tile([P, K, LC], BF16, tag="pr", name=f"pr{g}_{oc}")
                base = co[g][:]
                win = bass.AP(base.tensor, base.offset + oc * LC,
                              [list(base.ap)[0], [1, K], [1, LC]])
                nc.vector.tensor_mul(pr[:], win, kx[:])
                nc.vector.tensor_add(pr[:, 0:4, :], pr[:, 0:4, :], pr[:, 4:8, :])
                nc.vector.tensor_add(pr[:, 0:2, :], pr[:, 0:2, :], pr[:, 2:4, :])
                nc.vector.tensor_add(pr[:, 0, :], pr[:, 0, :], pr[:, 1, :])
                nc.vector.tensor_add(pr[:, 0, :], pr[:, 0, :], pr[:, 8, :])
                nc.vector.tensor_mul(out_cl[g][:, oc * LC : (oc + 1) * LC],
                                     pr[:, 0, :],
                                     recip_b[g][:, oc * LC : (oc + 1) * LC])

        for g in range(NCT):
            nc.sync.dma_start(out_d[g * P : (g + 1) * P, :], out_cl[g][:])


_NC_CACHE = None


def _build():
    global _NC_CACHE
    if _NC_CACHE is None:
        nc = bacc.Bacc("TRN2", target_bir_lowering=False, debug=False)
        with tile.TileContext(nc) as tc:
            _emit(nc, tc)
        nc.compile()
        _NC_CACHE = nc
    return _NC_CACHE


def _host_inputs(hidden_states, W_q, dw, pw, W_ck, W_co, b_q, b_co, sep_bias, b_ck):
    bf = ml_dtypes.bfloat16
    wqco = np.concatenate([W_q, W_co], axis=1).astype(bf)
    pwT = np.ascontiguousarray(pw.T).astype(bf)
    wck = W_ck.astype(bf)
    dws = np.asarray(dw, np.float32).reshape(C, K)
    bqco = np.concatenate([b_q.reshape(NCT, P), b_co.reshape(NCT, P)], axis=0)
    bqco = np.ascontiguousarray(bqco.T).astype(np.float32)  # [128, 12], col = otile
    bsep = np.ascontiguousarray(sep_bias.reshape(NCT, P).T).astype(np.float32)
    bck = np.asarray(b_ck, np.float32).reshape(HK, 1)
    shared = {"wqco": wqco, "pwT": pwT, "wck": wck, "dws": dws,
              "bqco": bqco, "bsep": bsep, "bck": bck}
    maps = []
    for b in range(B):
        xT = np.ascontiguousarray(np.asarray(hidden_states[b]).T).astype(bf)
        m = dict(shared)
        m["xT"] = xT
        maps.append(m)
    return maps


def kernel(hidden_states, W_q, b_q, dw, pw, sep_bias, W_ck, b_ck, W_co, b_co):
    hidden_states = np.asarray(hidden_states, np.float32)
    nc = _build()
    maps = _host_inputs(hidden_states, np.asarray(W_q, np.float32),
                        np.asarray(dw, np.float32), np.asarray(pw, np.float32),
                        np.asarray(W_ck, np.float32), np.asarray(W_co, np.float32),
                        np.asarray(b_q, np.float32), np.asarray(b_co, np.float32),
                        np.asarray(sep_bias, np.float32), np.asarray(b_ck, np.float32))
    res = run_bass_kernel_spmd(nc, maps, list(range(B)))
    out = np.empty((B, L, C), np.float32)
    for b in range(B):
        out[b] = np.asarray(res.results[b]["out"]).T.astype(np.float32)
    return out
